# revision 21
# baseline (speedup 1.0000x reference)
"""CIF (Continuous Integrate-and-Fire) segment-reduce kernel for Trainium2 (8 NeuronCores).

Structure (B=32, T=2000, H=512, L_OUT=250, threshold=0.95), data-parallel
over B: 4 examples per core.

  * The scan over T is a recurrence ONLY in the scalar integrator driven by
    `alphas` [B,T] (256 KB).  We replicate the reference's sequential fp32
    arithmetic exactly on the host (same op order -> bit-identical fire
    decisions); each step t then contributes to at most two output slots:
      - no fire:  alpha_t             -> slot n_prev
      - fire:     1 - integrate_{t-1} -> slot n_prev,
                  alpha_t - dist_comp -> slot n_prev+1
    Contributions to slots >= min(#fires, L_OUT) are dropped, matching the
    reference's gather/valid masking.

  * The heavy part, out[b,l] = sum_t W[b,l,t] * hidden[b,t], is a banded
    matmul: since sum(alphas) == 250 per row, the band drifts exactly
    15.625 slots per 125-step chunk (deviation is a Brownian bridge,
    sigma ~1.6 slots; the builder asserts the actual band fits each
    chunk's 40-slot window).  Weights upload compactly as [125, 16, 40]
    fp16; the DVE zeroes a [125, 18, 128] piece tile and scatters each
    chunk's band to its window offset (PE tile-position rules only allow
    128-wide outputs at PSUM partition 0, so every matmul is a full-panel
    accumulate; chunks 7-8 straddle the two 128-slot panels and get two
    pieces).  Per example: 18 fp16 matmuls W_i[125,128]^T @ h_c[125,512]
    into 2 PSUM banks, start=True on each panel's first piece.  All 8 PSUM
    banks hold the 4 examples' panels concurrently; DVE casts finished
    panels to fp16 staging.

  * DMA: everything rides the gpsimd SWDGE queue, which sprays each DMA's
    descriptors over the 16 SDMA engines in 25-descriptor ring chunks with
    a sliding start ring -- a stream of similar DMAs self-balances (v1's
    killer: HWDGE rings pinned W + outputs onto 2 engines that also carried
    SWDGE -> 107us busy of a 118us kernel).  hidden is host-cast to fp16
    and host-transposed to [125, 16, 512] (partition p, chunk c = step
    125c+p, 4 KB lines), uploaded in 4 segment DMAs per example (2/4/5/5
    chunks) so the PE chases the stream; outputs leave as fp16 on the SWDGE
    tail and the host casts back to fp32 (adds ~2.4e-4 rel error).

  Per-core traffic ~ 8.4 MB hidden + 0.64 MB W + 1 MB out.  The core
  sustains only ~200-230 GB/s aggregate DMA regardless of descriptor mix
  (a DMA util throttle caps ~50%), so the ~50 us stream is the floor and
  the 72 fp16 matmuls (~45 us at the PE's sustained 1.2 GHz) hide inside
  it: ~70 us total vs the 118 us baseline.
"""

import numpy as np

B, T, H = 32, 2000, 512
L_OUT = 250
N_CORES = 8
EX_PER_CORE = B // N_CORES      # 4
NCH = 16                        # 125-step chunks per example
KC = T // NCH                   # 125
# hidden segments per example (chunk ranges): a small first segment gets the
# PE started early; later ones sized so the tensor engine chases the stream
SEGS = [list(range(0, 3)), list(range(3, 9)), list(range(9, 16))]

# Band window (40 slots, arbitrary offset — the DVE expansion places it at
# any byte offset of the 128-wide piece tile) per chunk; nominal band of
# chunk c is [15.625c, 15.625(c+1)] +- Brownian bridge (sigma ~1.6 slots).
WB = 40
OFF = [min(max(round(15.625 * (_c + 0.5)) - 20, 0), 256 - 40) for _c in range(NCH)]
# PE tile-position rules force matmul output base partition 0 for >64-wide
# outputs, so each piece is a full 128-wide panel matmul (lhsT = a 128-wide
# SBUF weight tile that DVE assembles from the compact 64-wide upload).
# Chunks 7-8 straddle the panel boundary and contribute two pieces.
PIECES = [(c, p) for c in range(NCH) for p in range(2)
          if (p == 0 and OFF[c] < 128) or (p == 1 and OFF[c] + WB > 128)]
NMM = len(PIECES)               # 18
LAST_H0 = 8
LAST_H1 = 15

_PROGRAM = None        # cached compiled Bass program
LAST_RESULT = None     # BassKernelResults of the most recent run (introspection)
RUN_KWARGS = {}        # extra kwargs for run_bass_kernel_spmd (e.g. trace=True)


def _host_scan_weights(alphas: np.ndarray):
    """Replicates the reference scan's fp32 arithmetic exactly.

    Returns (wa, Ai, wb, Bi, ntot): per-step primary weight/slot, secondary
    (fire-only) weight/slot, and total fires per row.
    """
    a = np.ascontiguousarray(alphas, dtype=np.float32)
    Bb, Tt = a.shape
    ONE = np.float32(1.0)
    TH = np.float32(0.95)
    integrate = np.zeros(Bb, np.float32)
    n = np.zeros(Bb, np.int32)
    wa = np.empty((Bb, Tt), np.float32)
    wb = np.zeros((Bb, Tt), np.float32)
    Ai = np.empty((Bb, Tt), np.int32)
    Bi = np.empty((Bb, Tt), np.int32)
    for t in range(Tt):
        al = a[:, t]
        dist = ONE - integrate          # distribution_completion (fp32)
        integ = integrate + al          # fp32, same single add as reference
        f = integ > TH
        cur = np.where(f, dist, al)
        wa[:, t] = cur
        Ai[:, t] = n                    # n_prev
        wb[:, t] = np.where(f, al - cur, np.float32(0.0))
        Bi[:, t] = n + 1
        n = n + f
        integrate = np.where(f, integ - ONE, integ)  # exact subtract (Sterbenz)
    return wa, Ai, wb, Bi, n


def _build_weights(alphas: np.ndarray) -> np.ndarray:
    """Returns W [B, KC, NCH, WB] float16 banded weights (row p of chunk c =
    step 125c+p, col w = slot OFF[c]+w)."""
    wa, Ai, wb, Bi, ntot = _host_scan_weights(alphas)
    lim = np.minimum(ntot, L_OUT)[:, None].astype(np.int32)
    wa = np.where(Ai < lim, wa, np.float32(0.0))
    wb = np.where(Bi < lim, wb, np.float32(0.0))

    LPAD = 256
    Wd = np.zeros((B, T, LPAD), np.float32)
    bi = np.arange(B)[:, None]
    ti = np.arange(T)[None, :]
    Wd[bi, ti, np.minimum(Bi, LPAD - 1)] = wb
    Wd[bi, ti, np.minimum(Ai, LPAD - 1)] = wa

    Wc = Wd.reshape(B, NCH, KC, LPAD)
    W = np.empty((B, KC, NCH, WB), np.float16)
    for c in range(NCH):
        o = OFF[c]
        if Wc[:, c, :, :o].any() or Wc[:, c, :, o + WB:].any():
            raise AssertionError(f"chunk {c}: band mass outside window [{o},{o + WB})")
        W[:, :, c, :] = Wc[:, c, :, o:o + WB]
    return np.ascontiguousarray(W)


def _build_program():
    """Builds + compiles the per-core Bass/Tile program (SPMD, shared)."""
    import concourse.bacc as bacc
    import concourse.mybir as mybir
    import concourse.tile as tile

    nc = bacc.Bacc("TRN2", target_bir_lowering=False, debug=False, num_devices=N_CORES)
    hid = nc.dram_tensor(
        "hidden_sh", [EX_PER_CORE, KC, NCH, H], mybir.dt.float16,
        kind="ExternalInput"
    )
    wdr = nc.dram_tensor(
        "w_sh", [EX_PER_CORE, KC, NCH, WB], mybir.dt.float16, kind="ExternalInput"
    )
    out = nc.dram_tensor(
        "out_sh", [EX_PER_CORE, L_OUT, H], mybir.dt.float16, kind="ExternalOutput"
    )

    f32 = mybir.dt.float32
    f16 = mybir.dt.float16
    E = EX_PER_CORE
    with tile.TileContext(nc) as tc:
        with (
            tc.tile_pool(name="hp", bufs=E) as hpool,
            tc.tile_pool(name="wp", bufs=E) as wpool,
            tc.tile_pool(name="we", bufs=E) as wepool,
            tc.tile_pool(name="op", bufs=E) as opool,
            tc.tile_pool(name="psp", bufs=2 * E, space="PSUM") as pspool,
        ):
            panels = [
                [pspool.tile([128, H], f32, name=f"ps{e}_{h}", tag="ps") for h in range(2)]
                for e in range(E)
            ]
            w64 = [wpool.tile([KC, NCH, WB], f16, name=f"w64_{e}", tag="w64") for e in range(E)]
            w128 = [wepool.tile([KC, NMM, 128], f16, name=f"w128_{e}", tag="w128") for e in range(E)]
            # hidden per example in segment tiles for load->matmul
            # pipelining at ~0.25-0.65 MB granularity
            ht = [
                [hpool.tile([KC, len(seg), H], f16, name=f"h{e}_{si}", tag=f"h{si}")
                 for si, seg in enumerate(SEGS)]
                for e in range(E)
            ]
            oa = [opool.tile([128, H], f16, name=f"oa{e}", tag="oa") for e in range(E)]
            ob = [opool.tile([L_OUT - 128, H], f16, name=f"ob{e}", tag="ob") for e in range(E)]

            piece_idx = {cp: i for i, cp in enumerate(PIECES)}
            # hidden owns the SWDGE queue (the 25-descriptor ring-chunking
            # spreads every DMA across all 16 engines); the compact W rides
            # the sync HWDGE ring (rings 0-4, idle while the stream ramps) so
            # its bytes don't eat the SWDGE budget.
            for e in range(E):
                nc.sync.dma_start(w64[e][:], wdr[e])
                nc.gpsimd.dma_start(ht[e][0][:], hid[e, :, SEGS[0][0]:SEGS[0][-1] + 1, :])
            for si, seg in list(enumerate(SEGS))[1:]:
                for e in range(E):
                    nc.gpsimd.dma_start(
                        ht[e][si][:], hid[e, :, seg[0]:seg[-1] + 1, :]
                    )
            # DVE zeroes each 128-wide weight tile then immediately scatters
            # that example's 64-wide bands into the piece windows, so example
            # 0's weights are ready ~4x sooner than zero-everything-first.
            for e in range(E):
                nc.vector.memset(w128[e][:], 0.0)
                for c in range(NCH):
                    o = OFF[c]
                    if o + WB <= 128 or o >= 128:
                        i = piece_idx[(c, 0 if o + WB <= 128 else 1)]
                        lo = o - (128 if o >= 128 else 0)
                        nc.vector.tensor_copy(
                            w128[e][:, i, lo:lo + WB], w64[e][:, c, :]
                        )
                    else:
                        n0 = 128 - o
                        i0, i1 = piece_idx[(c, 0)], piece_idx[(c, 1)]
                        nc.vector.tensor_copy(w128[e][:, i0, o:128], w64[e][:, c, 0:n0])
                        nc.vector.tensor_copy(
                            w128[e][:, i1, 0:WB - n0], w64[e][:, c, n0:WB]
                        )

            for si, seg in enumerate(SEGS):
                for e in range(E):
                    for c in seg:
                        for p in range(2):
                            if (c, p) not in piece_idx:
                                continue
                            start = (p == 0 and c == 0) or (p == 1 and c == 7)
                            stop = (p == 0 and c == LAST_H0) or (p == 1 and c == LAST_H1)
                            nc.tensor.matmul(
                                panels[e][p][:],
                                w128[e][:, piece_idx[(c, p)], :],
                                ht[e][si][:, c - seg[0], :],
                                start=start, stop=stop,
                            )
                    if seg[0] <= LAST_H0 <= seg[-1]:
                        # half0 complete: out rows 0-127 (slots 0-127)
                        nc.vector.tensor_copy(oa[e][:], panels[e][0][:])
                    if si == len(SEGS) - 1:
                        # half1 complete: rows 128-249 (slots 128-249)
                        nc.vector.tensor_copy(ob[e][:], panels[e][1][0:L_OUT - 128, :])
            # outputs trail on the two HWDGE rings (idle by then), keeping
            # their bytes off the SWDGE stream entirely
            for e in range(E):
                nc.scalar.dma_start(out[e, 0:128, :], oa[e][:])
                nc.sync.dma_start(out[e, 128:L_OUT, :], ob[e][:])
    nc.compile()
    return nc


def kernel(hidden: np.ndarray, alphas: np.ndarray) -> np.ndarray:
    global _PROGRAM, LAST_RESULT
    from concourse.bass_utils import run_bass_kernel_spmd

    hidden = np.asarray(hidden)
    alphas = np.ascontiguousarray(np.asarray(alphas), dtype=np.float32)
    assert hidden.shape == (B, T, H) and alphas.shape == (B, T)

    # [B, T, H] -> [B, 125, 16, H] fp16: (p, c) holds step 125c+p
    hid16 = np.ascontiguousarray(
        hidden.reshape(B, NCH, KC, H).transpose(0, 2, 1, 3).astype(np.float16)
    )
    W = _build_weights(alphas)

    if _PROGRAM is None:
        _PROGRAM = _build_program()
    nc = _PROGRAM

    in_maps = [
        {
            "hidden_sh": hid16[i * EX_PER_CORE:(i + 1) * EX_PER_CORE],
            "w_sh": W[i * EX_PER_CORE:(i + 1) * EX_PER_CORE],
        }
        for i in range(N_CORES)
    ]
    res = run_bass_kernel_spmd(nc, in_maps, list(range(N_CORES)), **RUN_KWARGS)
    LAST_RESULT = res
    outs = [np.asarray(r["out_sh"]) for r in res.results]
    return np.concatenate(outs, axis=0).astype(np.float32)


# revision 22
# speedup vs baseline: 1.0508x; 1.0508x over previous
"""CIF (Continuous Integrate-and-Fire) segment-reduce kernel for Trainium2 (8 NeuronCores).

Structure (B=32, T=2000, H=512, L_OUT=250, threshold=0.95), data-parallel
over B: 4 examples per core.

  * The scan over T is a recurrence ONLY in the scalar integrator driven by
    `alphas` [B,T] (256 KB).  We replicate the reference's sequential fp32
    arithmetic exactly on the host (same op order -> bit-identical fire
    decisions); each step t then contributes to at most two output slots:
      - no fire:  alpha_t             -> slot n_prev
      - fire:     1 - integrate_{t-1} -> slot n_prev,
                  alpha_t - dist_comp -> slot n_prev+1
    Contributions to slots >= min(#fires, L_OUT) are dropped, matching the
    reference's gather/valid masking.

  * The heavy part, out[b,l] = sum_t W[b,l,t] * hidden[b,t], is a banded
    matmul: since sum(alphas) == 250 per row, the band drifts exactly
    15.625 slots per 125-step chunk (deviation is a Brownian bridge,
    sigma ~1.6 slots; the builder asserts the actual band fits each
    chunk's 40-slot window).  Weights upload compactly as [125, 16, 40]
    fp16; the DVE zeroes a [125, 18, 128] piece tile and scatters each
    chunk's band to its window offset (PE tile-position rules only allow
    128-wide outputs at PSUM partition 0, so every matmul is a full-panel
    accumulate; chunks 7-8 straddle the two 128-slot panels and get two
    pieces).  Per example: 18 fp16 matmuls W_i[125,128]^T @ h_c[125,512]
    into 2 PSUM banks, start=True on each panel's first piece.  All 8 PSUM
    banks hold the 4 examples' panels concurrently; DVE casts finished
    panels to fp16 staging.

  * DMA: everything rides the gpsimd SWDGE queue, which sprays each DMA's
    descriptors over the 16 SDMA engines in 25-descriptor ring chunks with
    a sliding start ring -- a stream of similar DMAs self-balances (v1's
    killer: HWDGE rings pinned W + outputs onto 2 engines that also carried
    SWDGE -> 107us busy of a 118us kernel).  hidden is host-cast to fp16
    and host-transposed to [125, 16, 512] (partition p, chunk c = step
    125c+p, 4 KB lines), uploaded in 4 segment DMAs per example (2/4/5/5
    chunks) so the PE chases the stream; outputs leave as fp16 on the SWDGE
    tail and the host casts back to fp32 (adds ~2.4e-4 rel error).

  Per-core traffic ~ 8.4 MB hidden + 0.64 MB W + 1 MB out.  The core
  sustains only ~200-230 GB/s aggregate DMA regardless of descriptor mix
  (a DMA util throttle caps ~50%), so the ~50 us stream is the floor and
  the 72 fp16 matmuls (~45 us at the PE's sustained 1.2 GHz) hide inside
  it: ~70 us total vs the 118 us baseline.
"""

import numpy as np

B, T, H = 32, 2000, 512
L_OUT = 250
N_CORES = 8
EX_PER_CORE = B // N_CORES      # 4
NCH = 16                        # 125-step chunks per example
KC = T // NCH                   # 125
# hidden segments per example (chunk ranges): a small first segment gets the
# PE started early; later ones sized so the tensor engine chases the stream
SEGS = [list(range(0, 2)), list(range(2, 6)), list(range(6, 11)), list(range(11, 16))]

# Band window (40 slots, arbitrary offset — the DVE expansion places it at
# any byte offset of the 128-wide piece tile) per chunk; nominal band of
# chunk c is [15.625c, 15.625(c+1)] +- Brownian bridge (sigma ~1.6 slots).
WB = 40
OFF = [min(max(round(15.625 * (_c + 0.5)) - 20, 0), 256 - 40) for _c in range(NCH)]
# PE tile-position rules force matmul output base partition 0 for >64-wide
# outputs, so each piece is a full 128-wide panel matmul (lhsT = a 128-wide
# SBUF weight tile that DVE assembles from the compact 64-wide upload).
# Chunks 7-8 straddle the panel boundary and contribute two pieces.
PIECES = [(c, p) for c in range(NCH) for p in range(2)
          if (p == 0 and OFF[c] < 128) or (p == 1 and OFF[c] + WB > 128)]
NMM = len(PIECES)               # 18
LAST_H0 = 8
LAST_H1 = 15

_PROGRAM = None        # cached compiled Bass program
LAST_RESULT = None     # BassKernelResults of the most recent run (introspection)
RUN_KWARGS = {}        # extra kwargs for run_bass_kernel_spmd (e.g. trace=True)


def _host_scan_weights(alphas: np.ndarray):
    """Replicates the reference scan's fp32 arithmetic exactly.

    Returns (wa, Ai, wb, Bi, ntot): per-step primary weight/slot, secondary
    (fire-only) weight/slot, and total fires per row.
    """
    a = np.ascontiguousarray(alphas, dtype=np.float32)
    Bb, Tt = a.shape
    ONE = np.float32(1.0)
    TH = np.float32(0.95)
    integrate = np.zeros(Bb, np.float32)
    n = np.zeros(Bb, np.int32)
    wa = np.empty((Bb, Tt), np.float32)
    wb = np.zeros((Bb, Tt), np.float32)
    Ai = np.empty((Bb, Tt), np.int32)
    Bi = np.empty((Bb, Tt), np.int32)
    for t in range(Tt):
        al = a[:, t]
        dist = ONE - integrate          # distribution_completion (fp32)
        integ = integrate + al          # fp32, same single add as reference
        f = integ > TH
        cur = np.where(f, dist, al)
        wa[:, t] = cur
        Ai[:, t] = n                    # n_prev
        wb[:, t] = np.where(f, al - cur, np.float32(0.0))
        Bi[:, t] = n + 1
        n = n + f
        integrate = np.where(f, integ - ONE, integ)  # exact subtract (Sterbenz)
    return wa, Ai, wb, Bi, n


def _build_weights(alphas: np.ndarray) -> np.ndarray:
    """Returns W [B, KC, NCH, WB] float16 banded weights (row p of chunk c =
    step 125c+p, col w = slot OFF[c]+w)."""
    wa, Ai, wb, Bi, ntot = _host_scan_weights(alphas)
    lim = np.minimum(ntot, L_OUT)[:, None].astype(np.int32)
    wa = np.where(Ai < lim, wa, np.float32(0.0))
    wb = np.where(Bi < lim, wb, np.float32(0.0))

    LPAD = 256
    Wd = np.zeros((B, T, LPAD), np.float32)
    bi = np.arange(B)[:, None]
    ti = np.arange(T)[None, :]
    Wd[bi, ti, np.minimum(Bi, LPAD - 1)] = wb
    Wd[bi, ti, np.minimum(Ai, LPAD - 1)] = wa

    Wc = Wd.reshape(B, NCH, KC, LPAD)
    W = np.empty((B, KC, NCH, WB), np.float16)
    for c in range(NCH):
        o = OFF[c]
        if Wc[:, c, :, :o].any() or Wc[:, c, :, o + WB:].any():
            raise AssertionError(f"chunk {c}: band mass outside window [{o},{o + WB})")
        W[:, :, c, :] = Wc[:, c, :, o:o + WB]
    return np.ascontiguousarray(W)


def _build_program():
    """Builds + compiles the per-core Bass/Tile program (SPMD, shared)."""
    import concourse.bacc as bacc
    import concourse.mybir as mybir
    import concourse.tile as tile

    nc = bacc.Bacc("TRN2", target_bir_lowering=False, debug=False, num_devices=N_CORES)
    hid = nc.dram_tensor(
        "hidden_sh", [EX_PER_CORE, KC, NCH, H], mybir.dt.float16,
        kind="ExternalInput"
    )
    wdr = nc.dram_tensor(
        "w_sh", [EX_PER_CORE, KC, NCH, WB], mybir.dt.float16, kind="ExternalInput"
    )
    out = nc.dram_tensor(
        "out_sh", [EX_PER_CORE, L_OUT, H], mybir.dt.float16, kind="ExternalOutput"
    )

    f32 = mybir.dt.float32
    f16 = mybir.dt.float16
    E = EX_PER_CORE
    with tile.TileContext(nc) as tc:
        with (
            tc.tile_pool(name="hp", bufs=E) as hpool,
            tc.tile_pool(name="wp", bufs=E) as wpool,
            tc.tile_pool(name="we", bufs=E) as wepool,
            tc.tile_pool(name="op", bufs=E) as opool,
            tc.tile_pool(name="psp", bufs=2 * E, space="PSUM") as pspool,
        ):
            panels = [
                [pspool.tile([128, H], f32, name=f"ps{e}_{h}", tag="ps") for h in range(2)]
                for e in range(E)
            ]
            w64 = [wpool.tile([KC, NCH, WB], f16, name=f"w64_{e}", tag="w64") for e in range(E)]
            w128 = [wepool.tile([KC, NMM, 128], f16, name=f"w128_{e}", tag="w128") for e in range(E)]
            # hidden per example in segment tiles for load->matmul
            # pipelining at ~0.25-0.65 MB granularity
            ht = [
                [hpool.tile([KC, len(seg), H], f16, name=f"h{e}_{si}", tag=f"h{si}")
                 for si, seg in enumerate(SEGS)]
                for e in range(E)
            ]
            oa = [opool.tile([128, H], f16, name=f"oa{e}", tag="oa") for e in range(E)]
            ob = [opool.tile([L_OUT - 128, H], f16, name=f"ob{e}", tag="ob") for e in range(E)]

            piece_idx = {cp: i for i, cp in enumerate(PIECES)}
            # Everything rides SWDGE: its 25-descriptor ring-chunking with a
            # sliding start ring self-balances a uniform DMA stream across
            # all 16 engines, which beats parking W/out on the HWDGE rings
            # (those pin to engines 64-68 and unbalance the stream).
            for e in range(E):
                nc.gpsimd.dma_start(w64[e][:], wdr[e])
                nc.gpsimd.dma_start(ht[e][0][:], hid[e, :, SEGS[0][0]:SEGS[0][-1] + 1, :])
            for si, seg in list(enumerate(SEGS))[1:]:
                for e in range(E):
                    nc.gpsimd.dma_start(
                        ht[e][si][:], hid[e, :, seg[0]:seg[-1] + 1, :]
                    )
            # DVE zeroes each 128-wide weight tile then immediately scatters
            # that example's 64-wide bands into the piece windows, so example
            # 0's weights are ready ~4x sooner than zero-everything-first.
            for e in range(E):
                nc.vector.memset(w128[e][:], 0.0)
                for c in range(NCH):
                    o = OFF[c]
                    if o + WB <= 128 or o >= 128:
                        i = piece_idx[(c, 0 if o + WB <= 128 else 1)]
                        lo = o - (128 if o >= 128 else 0)
                        nc.vector.tensor_copy(
                            w128[e][:, i, lo:lo + WB], w64[e][:, c, :]
                        )
                    else:
                        n0 = 128 - o
                        i0, i1 = piece_idx[(c, 0)], piece_idx[(c, 1)]
                        nc.vector.tensor_copy(w128[e][:, i0, o:128], w64[e][:, c, 0:n0])
                        nc.vector.tensor_copy(
                            w128[e][:, i1, 0:WB - n0], w64[e][:, c, n0:WB]
                        )

            for si, seg in enumerate(SEGS):
                for e in range(E):
                    for c in seg:
                        for p in range(2):
                            if (c, p) not in piece_idx:
                                continue
                            start = (p == 0 and c == 0) or (p == 1 and c == 7)
                            stop = (p == 0 and c == LAST_H0) or (p == 1 and c == LAST_H1)
                            nc.tensor.matmul(
                                panels[e][p][:],
                                w128[e][:, piece_idx[(c, p)], :],
                                ht[e][si][:, c - seg[0], :],
                                start=start, stop=stop,
                            )
                    if seg[0] <= LAST_H0 <= seg[-1]:
                        # half0 complete: out rows 0-127 (slots 0-127)
                        nc.vector.tensor_copy(oa[e][:], panels[e][0][:])
                    if si == len(SEGS) - 1:
                        # half1 complete: rows 128-249 (slots 128-249)
                        nc.vector.tensor_copy(ob[e][:], panels[e][1][0:L_OUT - 128, :])
            # outputs trail on SWDGE (sliding window keeps all 16 engines even)
            for e in range(E):
                nc.gpsimd.dma_start(out[e, 0:128, :], oa[e][:])
                nc.gpsimd.dma_start(out[e, 128:L_OUT, :], ob[e][:])
    nc.compile()
    return nc


def kernel(hidden: np.ndarray, alphas: np.ndarray) -> np.ndarray:
    global _PROGRAM, LAST_RESULT
    from concourse.bass_utils import run_bass_kernel_spmd

    hidden = np.asarray(hidden)
    alphas = np.ascontiguousarray(np.asarray(alphas), dtype=np.float32)
    assert hidden.shape == (B, T, H) and alphas.shape == (B, T)

    # [B, T, H] -> [B, 125, 16, H] fp16: (p, c) holds step 125c+p
    hid16 = np.ascontiguousarray(
        hidden.reshape(B, NCH, KC, H).transpose(0, 2, 1, 3).astype(np.float16)
    )
    W = _build_weights(alphas)

    if _PROGRAM is None:
        _PROGRAM = _build_program()
    nc = _PROGRAM

    in_maps = [
        {
            "hidden_sh": hid16[i * EX_PER_CORE:(i + 1) * EX_PER_CORE],
            "w_sh": W[i * EX_PER_CORE:(i + 1) * EX_PER_CORE],
        }
        for i in range(N_CORES)
    ]
    res = run_bass_kernel_spmd(nc, in_maps, list(range(N_CORES)), **RUN_KWARGS)
    LAST_RESULT = res
    outs = [np.asarray(r["out_sh"]) for r in res.results]
    return np.concatenate(outs, axis=0).astype(np.float32)


# revision 23
# speedup vs baseline: 1.0609x; 1.0096x over previous
"""CIF (Continuous Integrate-and-Fire) segment-reduce kernel for Trainium2 (8 NeuronCores).

Structure (B=32, T=2000, H=512, L_OUT=250, threshold=0.95), data-parallel
over B: 4 examples per core.

  * The scan over T is a recurrence ONLY in the scalar integrator driven by
    `alphas` [B,T] (256 KB).  We replicate the reference's sequential fp32
    arithmetic exactly on the host (same op order -> bit-identical fire
    decisions); each step t then contributes to at most two output slots:
      - no fire:  alpha_t             -> slot n_prev
      - fire:     1 - integrate_{t-1} -> slot n_prev,
                  alpha_t - dist_comp -> slot n_prev+1
    Contributions to slots >= min(#fires, L_OUT) are dropped, matching the
    reference's gather/valid masking.

  * The heavy part, out[b,l] = sum_t W[b,l,t] * hidden[b,t], is a banded
    matmul: since sum(alphas) == 250 per row, the band drifts exactly
    15.625 slots per 125-step chunk (deviation is a Brownian bridge,
    sigma ~1.6 slots; the builder asserts the actual band fits each
    chunk's 40-slot window).  Weights upload compactly as [125, 16, 40]
    fp16; the DVE zeroes a [125, 18, 128] piece tile and scatters each
    chunk's band to its window offset (PE tile-position rules only allow
    128-wide outputs at PSUM partition 0, so every matmul is a full-panel
    accumulate; chunks 7-8 straddle the two 128-slot panels and get two
    pieces).  Per example: 18 fp16 matmuls W_i[125,128]^T @ h_c[125,512]
    into 2 PSUM banks, start=True on each panel's first piece.  All 8 PSUM
    banks hold the 4 examples' panels concurrently; DVE casts finished
    panels to fp16 staging.

  * DMA: everything rides the gpsimd SWDGE queue, which sprays each DMA's
    descriptors over the 16 SDMA engines in 25-descriptor ring chunks with
    a sliding start ring -- a stream of similar DMAs self-balances (v1's
    killer: HWDGE rings pinned W + outputs onto 2 engines that also carried
    SWDGE -> 107us busy of a 118us kernel).  hidden is host-cast to fp16
    and host-transposed to [125, 16, 512] (partition p, chunk c = step
    125c+p, 4 KB lines), uploaded in 4 segment DMAs per example (2/4/5/5
    chunks) so the PE chases the stream; outputs leave as fp16 on the SWDGE
    tail and the host casts back to fp32 (adds ~2.4e-4 rel error).

  Per-core traffic ~ 8.4 MB hidden + 0.64 MB W + 1 MB out.  The core
  sustains only ~200-230 GB/s aggregate DMA regardless of descriptor mix
  (a DMA util throttle caps ~50%), so the ~50 us stream is the floor and
  the 72 fp16 matmuls (~45 us at the PE's sustained 1.2 GHz) hide inside
  it: ~70 us total vs the 118 us baseline.
"""

import numpy as np

B, T, H = 32, 2000, 512
L_OUT = 250
N_CORES = 8
EX_PER_CORE = B // N_CORES      # 4
NCH = 16                        # 125-step chunks per example
KC = T // NCH                   # 125
# hidden segments per example (chunk ranges): a small first segment gets the
# PE started early; later ones sized so the tensor engine chases the stream
SEGS = [list(range(0, 2)), list(range(2, 6)), list(range(6, 11)),
        list(range(11, 14)), list(range(14, 16))]

# Band window (40 slots, arbitrary offset — the DVE expansion places it at
# any byte offset of the 128-wide piece tile) per chunk; nominal band of
# chunk c is [15.625c, 15.625(c+1)] +- Brownian bridge (sigma ~1.6 slots).
WB = 40
OFF = [min(max(round(15.625 * (_c + 0.5)) - 20, 0), 256 - 40) for _c in range(NCH)]
# PE tile-position rules force matmul output base partition 0 for >64-wide
# outputs, so each piece is a full 128-wide panel matmul (lhsT = a 128-wide
# SBUF weight tile that DVE assembles from the compact 64-wide upload).
# Chunks 7-8 straddle the panel boundary and contribute two pieces.
PIECES = [(c, p) for c in range(NCH) for p in range(2)
          if (p == 0 and OFF[c] < 128) or (p == 1 and OFF[c] + WB > 128)]
NMM = len(PIECES)               # 18
LAST_H0 = 8
LAST_H1 = 15

_PROGRAM = None        # cached compiled Bass program
LAST_RESULT = None     # BassKernelResults of the most recent run (introspection)
RUN_KWARGS = {}        # extra kwargs for run_bass_kernel_spmd (e.g. trace=True)


def _host_scan_weights(alphas: np.ndarray):
    """Replicates the reference scan's fp32 arithmetic exactly.

    Returns (wa, Ai, wb, Bi, ntot): per-step primary weight/slot, secondary
    (fire-only) weight/slot, and total fires per row.
    """
    a = np.ascontiguousarray(alphas, dtype=np.float32)
    Bb, Tt = a.shape
    ONE = np.float32(1.0)
    TH = np.float32(0.95)
    integrate = np.zeros(Bb, np.float32)
    n = np.zeros(Bb, np.int32)
    wa = np.empty((Bb, Tt), np.float32)
    wb = np.zeros((Bb, Tt), np.float32)
    Ai = np.empty((Bb, Tt), np.int32)
    Bi = np.empty((Bb, Tt), np.int32)
    for t in range(Tt):
        al = a[:, t]
        dist = ONE - integrate          # distribution_completion (fp32)
        integ = integrate + al          # fp32, same single add as reference
        f = integ > TH
        cur = np.where(f, dist, al)
        wa[:, t] = cur
        Ai[:, t] = n                    # n_prev
        wb[:, t] = np.where(f, al - cur, np.float32(0.0))
        Bi[:, t] = n + 1
        n = n + f
        integrate = np.where(f, integ - ONE, integ)  # exact subtract (Sterbenz)
    return wa, Ai, wb, Bi, n


def _build_weights(alphas: np.ndarray) -> np.ndarray:
    """Returns W [B, KC, NCH, WB] float16 banded weights (row p of chunk c =
    step 125c+p, col w = slot OFF[c]+w)."""
    wa, Ai, wb, Bi, ntot = _host_scan_weights(alphas)
    lim = np.minimum(ntot, L_OUT)[:, None].astype(np.int32)
    wa = np.where(Ai < lim, wa, np.float32(0.0))
    wb = np.where(Bi < lim, wb, np.float32(0.0))

    LPAD = 256
    Wd = np.zeros((B, T, LPAD), np.float32)
    bi = np.arange(B)[:, None]
    ti = np.arange(T)[None, :]
    Wd[bi, ti, np.minimum(Bi, LPAD - 1)] = wb
    Wd[bi, ti, np.minimum(Ai, LPAD - 1)] = wa

    Wc = Wd.reshape(B, NCH, KC, LPAD)
    W = np.empty((B, KC, NCH, WB), np.float16)
    for c in range(NCH):
        o = OFF[c]
        if Wc[:, c, :, :o].any() or Wc[:, c, :, o + WB:].any():
            raise AssertionError(f"chunk {c}: band mass outside window [{o},{o + WB})")
        W[:, :, c, :] = Wc[:, c, :, o:o + WB]
    return np.ascontiguousarray(W)


def _build_program():
    """Builds + compiles the per-core Bass/Tile program (SPMD, shared)."""
    import concourse.bacc as bacc
    import concourse.mybir as mybir
    import concourse.tile as tile

    nc = bacc.Bacc("TRN2", target_bir_lowering=False, debug=False, num_devices=N_CORES)
    hid = nc.dram_tensor(
        "hidden_sh", [EX_PER_CORE, KC, NCH, H], mybir.dt.float16,
        kind="ExternalInput"
    )
    wdr = nc.dram_tensor(
        "w_sh", [EX_PER_CORE, KC, NCH, WB], mybir.dt.float16, kind="ExternalInput"
    )
    out = nc.dram_tensor(
        "out_sh", [EX_PER_CORE, L_OUT, H], mybir.dt.float16, kind="ExternalOutput"
    )

    f32 = mybir.dt.float32
    f16 = mybir.dt.float16
    E = EX_PER_CORE
    with tile.TileContext(nc) as tc:
        with (
            tc.tile_pool(name="hp", bufs=E) as hpool,
            tc.tile_pool(name="wp", bufs=E) as wpool,
            tc.tile_pool(name="we", bufs=E) as wepool,
            tc.tile_pool(name="op", bufs=E) as opool,
            tc.tile_pool(name="psp", bufs=2 * E, space="PSUM") as pspool,
        ):
            panels = [
                [pspool.tile([128, H], f32, name=f"ps{e}_{h}", tag="ps") for h in range(2)]
                for e in range(E)
            ]
            w64 = [wpool.tile([KC, NCH, WB], f16, name=f"w64_{e}", tag="w64") for e in range(E)]
            w128 = [wepool.tile([KC, NMM, 128], f16, name=f"w128_{e}", tag="w128") for e in range(E)]
            # hidden per example in segment tiles for load->matmul
            # pipelining at ~0.25-0.65 MB granularity
            ht = [
                [hpool.tile([KC, len(seg), H], f16, name=f"h{e}_{si}", tag=f"h{si}")
                 for si, seg in enumerate(SEGS)]
                for e in range(E)
            ]
            oa = [opool.tile([128, H], f16, name=f"oa{e}", tag="oa") for e in range(E)]
            ob = [opool.tile([L_OUT - 128, H], f16, name=f"ob{e}", tag="ob") for e in range(E)]

            piece_idx = {cp: i for i, cp in enumerate(PIECES)}
            # Everything rides SWDGE: its 25-descriptor ring-chunking with a
            # sliding start ring self-balances a uniform DMA stream across
            # all 16 engines, which beats parking W/out on the HWDGE rings
            # (those pin to engines 64-68 and unbalance the stream).
            for e in range(E):
                nc.gpsimd.dma_start(w64[e][:], wdr[e])
                nc.gpsimd.dma_start(ht[e][0][:], hid[e, :, SEGS[0][0]:SEGS[0][-1] + 1, :])
            for si, seg in list(enumerate(SEGS))[1:]:
                for e in range(E):
                    nc.gpsimd.dma_start(
                        ht[e][si][:], hid[e, :, seg[0]:seg[-1] + 1, :]
                    )
            # DVE zeroes each 128-wide weight tile then immediately scatters
            # that example's 64-wide bands into the piece windows, so example
            # 0's weights are ready ~4x sooner than zero-everything-first.
            for e in range(E):
                nc.vector.memset(w128[e][:], 0.0)
                for c in range(NCH):
                    o = OFF[c]
                    if o + WB <= 128 or o >= 128:
                        i = piece_idx[(c, 0 if o + WB <= 128 else 1)]
                        lo = o - (128 if o >= 128 else 0)
                        nc.vector.tensor_copy(
                            w128[e][:, i, lo:lo + WB], w64[e][:, c, :]
                        )
                    else:
                        n0 = 128 - o
                        i0, i1 = piece_idx[(c, 0)], piece_idx[(c, 1)]
                        nc.vector.tensor_copy(w128[e][:, i0, o:128], w64[e][:, c, 0:n0])
                        nc.vector.tensor_copy(
                            w128[e][:, i1, 0:WB - n0], w64[e][:, c, n0:WB]
                        )

            for si, seg in enumerate(SEGS):
                for e in range(E):
                    for c in seg:
                        for p in range(2):
                            if (c, p) not in piece_idx:
                                continue
                            start = (p == 0 and c == 0) or (p == 1 and c == 7)
                            stop = (p == 0 and c == LAST_H0) or (p == 1 and c == LAST_H1)
                            nc.tensor.matmul(
                                panels[e][p][:],
                                w128[e][:, piece_idx[(c, p)], :],
                                ht[e][si][:, c - seg[0], :],
                                start=start, stop=stop,
                            )
                    if seg[0] <= LAST_H0 <= seg[-1]:
                        # half0 complete: out rows 0-127 (slots 0-127)
                        nc.vector.tensor_copy(oa[e][:], panels[e][0][:])
                    if si == len(SEGS) - 1:
                        # half1 complete: rows 128-249 (slots 128-249)
                        nc.vector.tensor_copy(ob[e][:], panels[e][1][0:L_OUT - 128, :])
            # outputs trail on SWDGE (sliding window keeps all 16 engines even)
            for e in range(E):
                nc.gpsimd.dma_start(out[e, 0:128, :], oa[e][:])
                nc.gpsimd.dma_start(out[e, 128:L_OUT, :], ob[e][:])
    nc.compile()
    return nc


def kernel(hidden: np.ndarray, alphas: np.ndarray) -> np.ndarray:
    global _PROGRAM, LAST_RESULT
    from concourse.bass_utils import run_bass_kernel_spmd

    hidden = np.asarray(hidden)
    alphas = np.ascontiguousarray(np.asarray(alphas), dtype=np.float32)
    assert hidden.shape == (B, T, H) and alphas.shape == (B, T)

    # [B, T, H] -> [B, 125, 16, H] fp16: (p, c) holds step 125c+p
    hid16 = np.ascontiguousarray(
        hidden.reshape(B, NCH, KC, H).transpose(0, 2, 1, 3).astype(np.float16)
    )
    W = _build_weights(alphas)

    if _PROGRAM is None:
        _PROGRAM = _build_program()
    nc = _PROGRAM

    in_maps = [
        {
            "hidden_sh": hid16[i * EX_PER_CORE:(i + 1) * EX_PER_CORE],
            "w_sh": W[i * EX_PER_CORE:(i + 1) * EX_PER_CORE],
        }
        for i in range(N_CORES)
    ]
    res = run_bass_kernel_spmd(nc, in_maps, list(range(N_CORES)), **RUN_KWARGS)
    LAST_RESULT = res
    outs = [np.asarray(r["out_sh"]) for r in res.results]
    return np.concatenate(outs, axis=0).astype(np.float32)
